# revision 3
# baseline (speedup 1.0000x reference)
"""Trainium2 Bass kernel: sparse attention with stochastic sigmoid gate.

reference semantics (B=4, N=1024, C=768, H=12):
    qkv  = x @ W_qkv.T + b_qkv                  -> q, k, v  [B,H,N,64]
    qk   = q @ k.T                              [B,H,N,N]
    attn_mean   = softmax(qk * 64**-0.5)
    uncertainty = sigmoid(qk)
    attn = attn_mean * (r > uncertainty)
    out  = (attn @ v) @ W_proj.T + b_proj
    returns (out, attn_mean, uncertainty)

Sharding: 8 cores = (batch b in 0..4) x (query-half qh in 0..2). Each core
computes, for its 512 query rows of batch b and ALL 12 heads: attn_mean,
uncertainty (sliced outputs) and the fully-projected out rows. The host only
slices inputs and concatenates/transposes outputs - no host arithmetic.
"""
import sys

if "/opt/trn_rl_repo" not in sys.path:
    sys.path.insert(0, "/opt/trn_rl_repo")

import numpy as np
from contextlib import ExitStack

B, N, C, H = 4, 1024, 768, 12
HD = C // H           # 64
Q = N // 2            # 512 query rows per core
NCORES = 8
SCALE = float(HD) ** -0.5

_CACHE = {}


def _build_program():
    import concourse.bacc as bacc
    import concourse.tile as tile
    from concourse import mybir
    from concourse.masks import make_identity

    F32 = mybir.dt.float32
    F32R = mybir.dt.float32r
    AF = mybir.ActivationFunctionType
    ALU = mybir.AluOpType

    nc = bacc.Bacc("TRN2", target_bir_lowering=False, debug=False,
                   num_devices=NCORES)

    xT = nc.dram_tensor("xT", [C, N], F32R, kind="ExternalInput").ap()
    xqT = nc.dram_tensor("xqT", [C, Q], F32R, kind="ExternalInput").ap()
    wqT = nc.dram_tensor("wqT", [C, 3 * C], F32R, kind="ExternalInput").ap()
    wpT = nc.dram_tensor("wpT", [HD, H, C], F32R, kind="ExternalInput").ap()
    bqv = nc.dram_tensor("bqv", [128, 12], F32, kind="ExternalInput").ap()
    bv = nc.dram_tensor("bv", [128, C], F32, kind="ExternalInput").ap()
    bp = nc.dram_tensor("bp", [128, 6], F32, kind="ExternalInput").ap()
    r_in = nc.dram_tensor("r", [H, Q, N], F32, kind="ExternalInput").ap()
    attn_o = nc.dram_tensor("attn_o", [H, Q, N], F32, kind="ExternalOutput").ap()
    unc_o = nc.dram_tensor("unc_o", [H, Q, N], F32, kind="ExternalOutput").ap()
    outT_o = nc.dram_tensor("outT_o", [C, Q], F32, kind="ExternalOutput").ap()

    with tile.TileContext(nc) as tc, ExitStack() as ctx:
        consts = ctx.enter_context(tc.tile_pool(name="consts", bufs=1))
        persist = ctx.enter_context(tc.tile_pool(name="persist", bufs=1))
        stats = ctx.enter_context(tc.tile_pool(name="stats", bufs=6))
        # PSUM: s0(2) + s1(2) + t(2) + av(2) = 8 banks
        psS = ctx.enter_context(tc.tile_pool(name="psS", bufs=1, space="PSUM"))
        psT = ctx.enter_context(tc.tile_pool(name="psT", bufs=2, space="PSUM"))
        psV = ctx.enter_context(tc.tile_pool(name="psV", bufs=2, space="PSUM"))

        # constants
        bq_sb = consts.tile([128, 12], F32)
        nc.sync.dma_start(out=bq_sb[:], in_=bqv)
        bv_sb = consts.tile([128, C], F32)
        nc.sync.dma_start(out=bv_sb[:], in_=bv)
        bp_sb = consts.tile([128, 6], F32)
        nc.sync.dma_start(out=bp_sb[:], in_=bp)
        idf = consts.tile([128, 128], F32)
        make_identity(nc, idf)
        ident = consts.tile([128, 128], F32R)
        nc.scalar.activation(ident[:], idf[:], AF.Identity, bias=0.0, scale=1.0)

        # persistent projections: q/k packed 2 heads per 128 partitions
        # (partition p: head 2m + p//64, dim p%64), v token-major.
        qT_sb = persist.tile([128, 6, Q], F32R)
        kT_sb = persist.tile([128, 6, N], F32R)
        v_sb = persist.tile([128, 8, C], F32R)
        outT_sb = persist.tile([64, H, Q], F32R)

        # ---- stage A: qkv projection ----
        with tc.tile_pool(name="stageA", bufs=1) as wpool:
            wq_sb = wpool.tile([128, 6, 3 * C], F32R)
            nc.sync.dma_start(out=wq_sb[:],
                              in_=wqT.rearrange("(ch p) f -> p ch f", p=128))
            xT_sb = wpool.tile([128, 6, N], F32R)
            nc.sync.dma_start(out=xT_sb[:],
                              in_=xT.rearrange("(ch p) n -> p ch n", p=128))
            xqT_sb = wpool.tile([128, 6, Q], F32R)
            nc.sync.dma_start(out=xqT_sb[:],
                              in_=xqT.rearrange("(ch p) n -> p ch n", p=128))

            for m in range(6):
                p = psT.tile([128, Q], F32, tag="t")
                for ch in range(6):
                    nc.tensor.matmul(p[:], wq_sb[:, ch, m * 128:(m + 1) * 128],
                                     xqT_sb[:, ch, :],
                                     start=(ch == 0), stop=(ch == 5))
                nc.scalar.activation(qT_sb[:, m, :], p[:], AF.Identity,
                                     bias=bq_sb[:, m:m + 1], scale=1.0)
                for nh in range(2):
                    p = psT.tile([128, 512], F32, tag="t")
                    for ch in range(6):
                        nc.tensor.matmul(
                            p[:], wq_sb[:, ch, C + m * 128:C + (m + 1) * 128],
                            xT_sb[:, ch, nh * 512:(nh + 1) * 512],
                            start=(ch == 0), stop=(ch == 5))
                    nc.scalar.activation(kT_sb[:, m, nh * 512:(nh + 1) * 512],
                                         p[:], AF.Identity,
                                         bias=bq_sb[:, 6 + m:7 + m], scale=1.0)
            for nt in range(8):
                for fh in range(2):
                    p = psT.tile([128, 384], F32, tag="t")
                    for ch in range(6):
                        nc.tensor.matmul(
                            p[:], xT_sb[:, ch, nt * 128:(nt + 1) * 128],
                            wq_sb[:, ch, 2 * C + fh * 384:2 * C + (fh + 1) * 384],
                            start=(ch == 0), stop=(ch == 5))
                    nc.vector.tensor_tensor(
                        out=v_sb[:, nt, fh * 384:(fh + 1) * 384], in0=p[:],
                        in1=bv_sb[:, fh * 384:(fh + 1) * 384], op=ALU.add)

        # ---- stage B: scores, softmax, sigmoid gate, transposes, AV ----
        work3 = ctx.enter_context(tc.tile_pool(name="work3", bufs=3))
        work2 = ctx.enter_context(tc.tile_pool(name="work2", bufs=2))
        attnTp = ctx.enter_context(tc.tile_pool(name="attnTp", bufs=2))
        for m in range(6):
            attnT = {}
            for j in range(2):
                attnT[2 * m + j] = attnTp.tile([128, 8, Q], F32R, tag="attnT",
                                               name="attnT")
            for qb in range(4):
                for j in range(2):
                    h = 2 * m + j
                    base = 64 * j
                    ps_s = psS.tile([128, N], F32, tag="s%d" % j)
                    for kh in range(2):
                        nc.tensor.matmul(
                            ps_s[:, kh * 512:(kh + 1) * 512],
                            qT_sb[base:base + 64, m, qb * 128:(qb + 1) * 128],
                            kT_sb[base:base + 64, m, kh * 512:(kh + 1) * 512],
                            tile_position=(base, 0))
                    r_t = work3.tile([128, N], F32, tag="r")
                    nc.sync.dma_start(out=r_t[:],
                                      in_=r_in[h, qb * 128:(qb + 1) * 128, :])
                    # uncertainty = sigmoid(S) = 0.5*tanh(S/2) + 0.5
                    # (tanh shares the exp ACT table set; Sigmoid does not)
                    unc_t = work3.tile([128, N], F32, tag="unc")
                    nc.scalar.activation(unc_t[:], ps_s[:], AF.Tanh,
                                         bias=0.0, scale=0.5)
                    nc.gpsimd.tensor_scalar(out=unc_t[:], in0=unc_t[:],
                                            scalar1=0.5, scalar2=0.5,
                                            op0=ALU.mult, op1=ALU.add)
                    nc.gpsimd.dma_start(
                        out=unc_o[h, qb * 128:(qb + 1) * 128, :], in_=unc_t[:])
                    # softmax without max-subtraction: |S*scale| <= ~2, exp is safe
                    exp_t = work2.tile([128, N], F32, tag="exp")
                    sum_t = stats.tile([128, 1], F32, tag="sum")
                    nc.scalar.activation(exp_t[:], ps_s[:], AF.Exp, bias=0.0,
                                         scale=SCALE, accum_out=sum_t[:])
                    rec_t = stats.tile([128, 1], F32, tag="rec")
                    nc.vector.reciprocal(rec_t[:], sum_t[:])
                    am_t = work2.tile([128, N], F32, tag="am")
                    nc.gpsimd.tensor_scalar_mul(am_t[:], exp_t[:], rec_t[:])
                    nc.gpsimd.dma_start(
                        out=attn_o[h, qb * 128:(qb + 1) * 128, :], in_=am_t[:])
                    # mask = r > uncertainty (in-place into r_t), then gate
                    nc.vector.tensor_tensor(out=r_t[:], in0=r_t[:],
                                            in1=unc_t[:], op=ALU.is_gt)
                    msk_t = work2.tile([128, N], F32R, tag="msk")
                    nc.vector.tensor_tensor(out=msk_t[:], in0=am_t[:],
                                            in1=r_t[:], op=ALU.mult)
                    # transpose masked attn into attnT[h][:, kc, qb*128:...]
                    for g in range(2):
                        ps_t = psT.tile([128, 512], F32R, tag="t")
                        for kk in range(4):
                            kc = g * 4 + kk
                            nc.tensor.transpose(
                                ps_t[:, kk * 128:(kk + 1) * 128],
                                msk_t[:, kc * 128:(kc + 1) * 128], ident[:])
                        nc.vector.tensor_copy(
                            attnT[h][:, g * 4:(g + 1) * 4,
                                     qb * 128:(qb + 1) * 128],
                            ps_t[:].rearrange("p (kk q) -> p kk q", kk=4))
            for j in range(2):
                h = 2 * m + j
                ps_o = psV.tile([64, Q], F32, tag="av")
                for kc in range(8):
                    nc.tensor.matmul(ps_o[:],
                                     v_sb[:, kc, h * 64:(h + 1) * 64],
                                     attnT[h][:, kc, :],
                                     start=(kc == 0), stop=(kc == 7))
                nc.vector.tensor_copy(outT_sb[:, h, :], ps_o[:])

        # ---- proj: out.T[e, q] = sum_f W_proj[e, f] * outT[f, q] + b_proj ----
        with tc.tile_pool(name="projp", bufs=1) as pp:
            wp_sb = pp.tile([64, H, C], F32R)
            nc.sync.dma_start(out=wp_sb[:], in_=wpT)
            for et in range(6):
                ps_p = psS.tile([128, Q], F32, tag="s0")
                for h in range(H):
                    nc.tensor.matmul(ps_p[:],
                                     wp_sb[:, h, et * 128:(et + 1) * 128],
                                     outT_sb[:, h, :],
                                     start=(h == 0), stop=(h == H - 1))
                of_t = work2.tile([128, Q], F32, tag="exp")
                nc.scalar.activation(of_t[:], ps_p[:], AF.Identity,
                                     bias=bp_sb[:, et:et + 1], scale=1.0)
                nc.gpsimd.dma_start(out=outT_o[et * 128:(et + 1) * 128, :],
                                    in_=of_t[:])

    nc.compile()
    return nc


def _get_program():
    if "nc" not in _CACHE:
        _CACHE["nc"] = _build_program()
    return _CACHE["nc"]


def _make_in_maps(x, r, W_qkv, b_qkv, W_proj, b_proj):
    x = np.ascontiguousarray(np.asarray(x, np.float32))
    r = np.asarray(r, np.float32)
    W_qkv = np.asarray(W_qkv, np.float32)
    b_qkv = np.asarray(b_qkv, np.float32)
    W_proj = np.asarray(W_proj, np.float32)
    b_proj = np.asarray(b_proj, np.float32)

    wqT_np = np.ascontiguousarray(W_qkv.T)                       # [C, 3C]
    wpT_np = np.ascontiguousarray(
        W_proj.T.reshape(H, HD, C).transpose(1, 0, 2))           # [HD, H, C]
    bqv_np = np.ascontiguousarray(b_qkv[:2 * C].reshape(12, 128).T)
    bv_np = np.ascontiguousarray(
        np.broadcast_to(b_qkv[2 * C:], (128, C)))
    bp_np = np.ascontiguousarray(b_proj.reshape(6, 128).T)

    in_maps = []
    for core in range(NCORES):
        b, qh = core // 2, core % 2
        qs = qh * Q
        xTb = np.ascontiguousarray(x[b].T)
        in_maps.append({
            "xT": xTb,
            "xqT": np.ascontiguousarray(xTb[:, qs:qs + Q]),
            "wqT": wqT_np, "wpT": wpT_np,
            "bqv": bqv_np, "bv": bv_np, "bp": bp_np,
            "r": np.ascontiguousarray(r[b, :, qs:qs + Q, :]),
        })
    return in_maps


def _assemble(results):
    out = np.empty((B, N, C), np.float32)
    attn = np.empty((B, H, N, N), np.float32)
    unc = np.empty((B, H, N, N), np.float32)
    for core in range(NCORES):
        b, qh = core // 2, core % 2
        qs = qh * Q
        rr = results[core]
        out[b, qs:qs + Q, :] = rr["outT_o"].T
        attn[b, :, qs:qs + Q, :] = rr["attn_o"]
        unc[b, :, qs:qs + Q, :] = rr["unc_o"]
    return out, attn, unc


def _run(in_maps, trace=False):
    from concourse.bass_utils import run_bass_kernel_spmd
    nc = _get_program()
    return run_bass_kernel_spmd(nc, in_maps, core_ids=list(range(NCORES)),
                                trace=trace)


def kernel(x, r, W_qkv, b_qkv, W_proj, b_proj):
    in_maps = _make_in_maps(x, r, W_qkv, b_qkv, W_proj, b_proj)
    res = _run(in_maps, trace=False)
    return _assemble(res.results)


def kernel_profiled(x, r, W_qkv, b_qkv, W_proj, b_proj):
    """Same as kernel() but captures an NTFF profile; returns (outputs, BassKernelResults)."""
    in_maps = _make_in_maps(x, r, W_qkv, b_qkv, W_proj, b_proj)
    res = _run(in_maps, trace=True)
    return _assemble(res.results), res


# revision 4
# speedup vs baseline: 2.4348x; 2.4348x over previous
"""Trainium2 Bass kernel: sparse attention with stochastic sigmoid gate.

reference semantics (B=4, N=1024, C=768, H=12):
    qkv  = x @ W_qkv.T + b_qkv                  -> q, k, v  [B,H,N,64]
    qk   = q @ k.T                              [B,H,N,N]
    attn_mean   = softmax(qk * 64**-0.5)
    uncertainty = sigmoid(qk)
    attn = attn_mean * (r > uncertainty)
    out  = (attn @ v) @ W_proj.T + b_proj
    returns (out, attn_mean, uncertainty)

Sharding: 8 cores = (batch b in 0..4) x (query-half qh in 0..2). Each core
computes, for its 512 query rows of batch b and ALL 12 heads: attn_mean,
uncertainty (sliced outputs) and the fully-projected out rows. The host only
slices inputs and concatenates/transposes outputs - no host arithmetic.
"""
import sys

if "/opt/trn_rl_repo" not in sys.path:
    sys.path.insert(0, "/opt/trn_rl_repo")

import numpy as np
from contextlib import ExitStack

B, N, C, H = 4, 1024, 768, 12
HD = C // H           # 64
Q = N // 2            # 512 query rows per core
NCORES = 8
SCALE = float(HD) ** -0.5

_CACHE = {}


def _build_program():
    import concourse.bacc as bacc
    import concourse.tile as tile
    from concourse import mybir
    from concourse.masks import make_identity

    F32 = mybir.dt.float32
    F32R = mybir.dt.float32r
    AF = mybir.ActivationFunctionType
    ALU = mybir.AluOpType

    nc = bacc.Bacc("TRN2", target_bir_lowering=False, debug=False,
                   num_devices=NCORES)

    xT = nc.dram_tensor("xT", [C, N], F32R, kind="ExternalInput").ap()
    xqT = nc.dram_tensor("xqT", [C, Q], F32R, kind="ExternalInput").ap()
    wqT = nc.dram_tensor("wqT", [C, 3 * C], F32R, kind="ExternalInput").ap()
    wpT = nc.dram_tensor("wpT", [HD, H, C], F32R, kind="ExternalInput").ap()
    bqv = nc.dram_tensor("bqv", [128, 12], F32, kind="ExternalInput").ap()
    bv = nc.dram_tensor("bv", [128, C], F32, kind="ExternalInput").ap()
    bp = nc.dram_tensor("bp", [128, 6], F32, kind="ExternalInput").ap()
    r_in = nc.dram_tensor("r", [H, Q, N], F32, kind="ExternalInput").ap()
    attn_o = nc.dram_tensor("attn_o", [H, Q, N], F32, kind="ExternalOutput").ap()
    unc_o = nc.dram_tensor("unc_o", [H, Q, N], F32, kind="ExternalOutput").ap()
    outT_o = nc.dram_tensor("outT_o", [C, Q], F32, kind="ExternalOutput").ap()

    with tile.TileContext(nc) as tc, ExitStack() as ctx:
        consts = ctx.enter_context(tc.tile_pool(name="consts", bufs=1))
        persist = ctx.enter_context(tc.tile_pool(name="persist", bufs=1))
        stats = ctx.enter_context(tc.tile_pool(name="stats", bufs=6))
        # PSUM: s0(2) + s1(2) + t(2) + av(2) = 8 banks
        psS = ctx.enter_context(tc.tile_pool(name="psS", bufs=1, space="PSUM"))
        psT = ctx.enter_context(tc.tile_pool(name="psT", bufs=2, space="PSUM"))
        psV = ctx.enter_context(tc.tile_pool(name="psV", bufs=2, space="PSUM"))

        # constants
        bq_sb = consts.tile([128, 12], F32)
        nc.sync.dma_start(out=bq_sb[:], in_=bqv)
        bv_sb = consts.tile([128, C], F32)
        nc.sync.dma_start(out=bv_sb[:], in_=bv)
        bp_sb = consts.tile([128, 6], F32)
        nc.sync.dma_start(out=bp_sb[:], in_=bp)
        idf = consts.tile([128, 128], F32)
        make_identity(nc, idf)
        ident = consts.tile([128, 128], F32R)
        nc.scalar.activation(ident[:], idf[:], AF.Identity, bias=0.0, scale=1.0)

        # persistent projections: q/k packed 2 heads per 128 partitions
        # (partition p: head 2m + p//64, dim p%64), v token-major.
        qT_sb = persist.tile([128, 6, Q], F32R)
        kT_sb = persist.tile([128, 6, N], F32R)
        v_sb = persist.tile([128, 8, C], F32R)
        outT_sb = persist.tile([64, H, Q], F32R)

        # ---- stage A: qkv projection ----
        with tc.tile_pool(name="stageA", bufs=1) as wpool:
            wq_sb = wpool.tile([128, 6, 3 * C], F32R)
            nc.sync.dma_start(out=wq_sb[:],
                              in_=wqT.rearrange("(ch p) f -> p ch f", p=128))
            xT_sb = wpool.tile([128, 6, N], F32R)
            nc.sync.dma_start(out=xT_sb[:],
                              in_=xT.rearrange("(ch p) n -> p ch n", p=128))
            xqT_sb = wpool.tile([128, 6, Q], F32R)
            nc.sync.dma_start(out=xqT_sb[:],
                              in_=xqT.rearrange("(ch p) n -> p ch n", p=128))

            for m in range(6):
                p = psT.tile([128, Q], F32, tag="t")
                for ch in range(6):
                    nc.tensor.matmul(p[:], wq_sb[:, ch, m * 128:(m + 1) * 128],
                                     xqT_sb[:, ch, :],
                                     start=(ch == 0), stop=(ch == 5))
                nc.scalar.activation(qT_sb[:, m, :], p[:], AF.Identity,
                                     bias=bq_sb[:, m:m + 1], scale=1.0)
                for nh in range(2):
                    p = psT.tile([128, 512], F32, tag="t")
                    for ch in range(6):
                        nc.tensor.matmul(
                            p[:], wq_sb[:, ch, C + m * 128:C + (m + 1) * 128],
                            xT_sb[:, ch, nh * 512:(nh + 1) * 512],
                            start=(ch == 0), stop=(ch == 5))
                    nc.scalar.activation(kT_sb[:, m, nh * 512:(nh + 1) * 512],
                                         p[:], AF.Identity,
                                         bias=bq_sb[:, 6 + m:7 + m], scale=1.0)
            for nt in range(8):
                for fh in range(2):
                    p = psT.tile([128, 384], F32, tag="t")
                    for ch in range(6):
                        nc.tensor.matmul(
                            p[:], xT_sb[:, ch, nt * 128:(nt + 1) * 128],
                            wq_sb[:, ch, 2 * C + fh * 384:2 * C + (fh + 1) * 384],
                            start=(ch == 0), stop=(ch == 5))
                    nc.vector.tensor_tensor(
                        out=v_sb[:, nt, fh * 384:(fh + 1) * 384], in0=p[:],
                        in1=bv_sb[:, fh * 384:(fh + 1) * 384], op=ALU.add)

        # ---- stage B: scores, softmax, sigmoid gate, transposes, AV ----
        work3 = ctx.enter_context(tc.tile_pool(name="work3", bufs=3))
        work2 = ctx.enter_context(tc.tile_pool(name="work2", bufs=2))
        attnTp = ctx.enter_context(tc.tile_pool(name="attnTp", bufs=2))
        for m in range(6):
            attnT = {}
            for j in range(2):
                attnT[2 * m + j] = attnTp.tile([128, 8, Q], F32R, tag="attnT",
                                               name="attnT")
            for qb in range(4):
                for j in range(2):
                    h = 2 * m + j
                    base = 64 * j
                    ps_s = psS.tile([128, N], F32, tag="s%d" % j)
                    for kh in range(2):
                        nc.tensor.matmul(
                            ps_s[:, kh * 512:(kh + 1) * 512],
                            qT_sb[base:base + 64, m, qb * 128:(qb + 1) * 128],
                            kT_sb[base:base + 64, m, kh * 512:(kh + 1) * 512],
                            tile_position=(base, 0))
                    r_t = work3.tile([128, N], F32, tag="r")
                    nc.sync.dma_start(out=r_t[:],
                                      in_=r_in[h, qb * 128:(qb + 1) * 128, :])
                    # uncertainty = sigmoid(S) = 0.5*tanh(S/2) + 0.5
                    # (tanh shares the exp ACT table set; Sigmoid does not)
                    unc_t = work3.tile([128, N], F32, tag="unc")
                    nc.scalar.activation(unc_t[:], ps_s[:], AF.Tanh,
                                         bias=0.0, scale=0.5)
                    nc.vector.tensor_scalar(out=unc_t[:], in0=unc_t[:],
                                            scalar1=0.5, scalar2=0.5,
                                            op0=ALU.mult, op1=ALU.add)
                    nc.gpsimd.dma_start(
                        out=unc_o[h, qb * 128:(qb + 1) * 128, :], in_=unc_t[:])
                    # softmax without max-subtraction: |S*scale| <= ~2, exp is safe
                    exp_t = work2.tile([128, N], F32, tag="exp")
                    sum_t = stats.tile([128, 1], F32, tag="sum")
                    nc.scalar.activation(exp_t[:], ps_s[:], AF.Exp, bias=0.0,
                                         scale=SCALE, accum_out=sum_t[:])
                    rec_t = stats.tile([128, 1], F32, tag="rec")
                    nc.vector.reciprocal(rec_t[:], sum_t[:])
                    am_t = work2.tile([128, N], F32, tag="am")
                    nc.scalar.activation(am_t[:], exp_t[:], AF.Copy,
                                         bias=0.0, scale=rec_t[:])
                    nc.gpsimd.dma_start(
                        out=attn_o[h, qb * 128:(qb + 1) * 128, :], in_=am_t[:])
                    # mask = r > uncertainty (in-place into r_t), then gate
                    nc.vector.tensor_tensor(out=r_t[:], in0=r_t[:],
                                            in1=unc_t[:], op=ALU.is_gt)
                    msk_t = work2.tile([128, N], F32R, tag="msk")
                    nc.vector.tensor_tensor(out=msk_t[:], in0=am_t[:],
                                            in1=r_t[:], op=ALU.mult)
                    # transpose masked attn into attnT[h][:, kc, qb*128:...]
                    for g in range(2):
                        ps_t = psT.tile([128, 512], F32R, tag="t")
                        for kk in range(4):
                            kc = g * 4 + kk
                            nc.tensor.transpose(
                                ps_t[:, kk * 128:(kk + 1) * 128],
                                msk_t[:, kc * 128:(kc + 1) * 128], ident[:])
                        nc.vector.tensor_copy(
                            attnT[h][:, g * 4:(g + 1) * 4,
                                     qb * 128:(qb + 1) * 128],
                            ps_t[:].rearrange("p (kk q) -> p kk q", kk=4))
            for j in range(2):
                h = 2 * m + j
                ps_o = psV.tile([64, Q], F32, tag="av")
                for kc in range(8):
                    nc.tensor.matmul(ps_o[:],
                                     v_sb[:, kc, h * 64:(h + 1) * 64],
                                     attnT[h][:, kc, :],
                                     start=(kc == 0), stop=(kc == 7))
                nc.vector.tensor_copy(outT_sb[:, h, :], ps_o[:])

        # ---- proj: out.T[e, q] = sum_f W_proj[e, f] * outT[f, q] + b_proj ----
        with tc.tile_pool(name="projp", bufs=1) as pp:
            wp_sb = pp.tile([64, H, C], F32R)
            nc.sync.dma_start(out=wp_sb[:], in_=wpT)
            for et in range(6):
                ps_p = psS.tile([128, Q], F32, tag="s0")
                for h in range(H):
                    nc.tensor.matmul(ps_p[:],
                                     wp_sb[:, h, et * 128:(et + 1) * 128],
                                     outT_sb[:, h, :],
                                     start=(h == 0), stop=(h == H - 1))
                of_t = work2.tile([128, Q], F32, tag="exp")
                nc.scalar.activation(of_t[:], ps_p[:], AF.Identity,
                                     bias=bp_sb[:, et:et + 1], scale=1.0)
                nc.gpsimd.dma_start(out=outT_o[et * 128:(et + 1) * 128, :],
                                    in_=of_t[:])

    nc.compile()
    return nc


def _get_program():
    if "nc" not in _CACHE:
        _CACHE["nc"] = _build_program()
    return _CACHE["nc"]


def _make_in_maps(x, r, W_qkv, b_qkv, W_proj, b_proj):
    x = np.ascontiguousarray(np.asarray(x, np.float32))
    r = np.asarray(r, np.float32)
    W_qkv = np.asarray(W_qkv, np.float32)
    b_qkv = np.asarray(b_qkv, np.float32)
    W_proj = np.asarray(W_proj, np.float32)
    b_proj = np.asarray(b_proj, np.float32)

    wqT_np = np.ascontiguousarray(W_qkv.T)                       # [C, 3C]
    wpT_np = np.ascontiguousarray(
        W_proj.T.reshape(H, HD, C).transpose(1, 0, 2))           # [HD, H, C]
    bqv_np = np.ascontiguousarray(b_qkv[:2 * C].reshape(12, 128).T)
    bv_np = np.ascontiguousarray(
        np.broadcast_to(b_qkv[2 * C:], (128, C)))
    bp_np = np.ascontiguousarray(b_proj.reshape(6, 128).T)

    in_maps = []
    for core in range(NCORES):
        b, qh = core // 2, core % 2
        qs = qh * Q
        xTb = np.ascontiguousarray(x[b].T)
        in_maps.append({
            "xT": xTb,
            "xqT": np.ascontiguousarray(xTb[:, qs:qs + Q]),
            "wqT": wqT_np, "wpT": wpT_np,
            "bqv": bqv_np, "bv": bv_np, "bp": bp_np,
            "r": np.ascontiguousarray(r[b, :, qs:qs + Q, :]),
        })
    return in_maps


def _assemble(results):
    out = np.empty((B, N, C), np.float32)
    attn = np.empty((B, H, N, N), np.float32)
    unc = np.empty((B, H, N, N), np.float32)
    for core in range(NCORES):
        b, qh = core // 2, core % 2
        qs = qh * Q
        rr = results[core]
        out[b, qs:qs + Q, :] = rr["outT_o"].T
        attn[b, :, qs:qs + Q, :] = rr["attn_o"]
        unc[b, :, qs:qs + Q, :] = rr["unc_o"]
    return out, attn, unc


def _run(in_maps, trace=False):
    from concourse.bass_utils import run_bass_kernel_spmd
    nc = _get_program()
    return run_bass_kernel_spmd(nc, in_maps, core_ids=list(range(NCORES)),
                                trace=trace)


def kernel(x, r, W_qkv, b_qkv, W_proj, b_proj):
    in_maps = _make_in_maps(x, r, W_qkv, b_qkv, W_proj, b_proj)
    res = _run(in_maps, trace=False)
    return _assemble(res.results)


def kernel_profiled(x, r, W_qkv, b_qkv, W_proj, b_proj):
    """Same as kernel() but captures an NTFF profile; returns (outputs, BassKernelResults)."""
    in_maps = _make_in_maps(x, r, W_qkv, b_qkv, W_proj, b_proj)
    res = _run(in_maps, trace=True)
    return _assemble(res.results), res
